# revision 6
# baseline (speedup 1.0000x reference)
"""Trainium2 Bass kernel for nn_EntropyLoss (256-bin histogram entropy diff).

Data-parallel over 8 NeuronCores: each core processes 8 of the 64 batch
entries of both tensors ([128, 32768] f32 per tensor per core).

Counting is cumulative: D_k = #{u >= k} for k = 0..255 where
u = (x+1)*128 computed exactly as the reference does (fl(x+1)*128 ==
fl(128x+128), both exact-rounded the same way).  Bins: c_k = D_k - D_{k+1}
(k<=254), c_255 = D_255 - #{x > 1}.  In-range/out-of-range and the
u == 256 edge cases (x in {1-2^-24, 1.0, 1+2^-23}) are exact under this
scheme (see derivation in comments below).

Engine split per 8192-col chunk:
  - VectorE: u = (x+1)*128 (dual-op tensor_scalar), E-pass #{x >= 1+2^-23},
    floor-fix j = r - (u < r) (custom fused DVE op), and is_ge(u, k)
    count passes with accum_out for k in the DVE boundary set.
  - ScalarE: r = int16 round-cast of u (Copy), and Sign(j - (k-0.5))
    cumulative passes with accum_out for k in the ACT boundary set
    (j is exact-integer bf16, so Sign never sees 0).
Host reconstructs exact integer counts from per-partition f32 accumulators
(all values < 2^24 so f32 accumulation is exact) and computes the entropy
diff with the reference's own fp32 formula.
"""

import os

import numpy as np

B, C, H, W = 64, 2, 512, 512
N_CORES = 8
P = 128
ELEMS_PER_CORE = (B // N_CORES) * C * H * W            # 4,194,304 per tensor
FREE = ELEMS_PER_CORE // P                             # 32,768
FQ = int(os.environ.get("ENT_FQ", "8192"))             # chunk width
J_I8 = os.environ.get("ENT_JI8", "1") == "1"           # j as offset int8
D_PSUM = os.environ.get("ENT_DPSUM", "1") == "1"       # DVE accums in PSUM
A_PSUM = os.environ.get("ENT_APSUM", "1") == "1"       # ACT accums in PSUM
NCH = FREE // FQ                                       # 4 chunks per tensor
NHALF = 2                                              # ACT counts on 16k halves
FH = FREE // NHALF                                     # 16384
NB = 256
N_DVE = int(os.environ.get("ENT_NDVE", "116"))         # boundaries 0..N_DVE-1 on DVE
N_ACT = NB - N_DVE                                     # boundaries N_DVE..255 on ACT
EPS = 1e-8
X_ABOVE_ONE = float(np.float32(1.0) + np.float32(2.0 ** -23))

# accumulator column layouts
N_DVE_J = N_DVE - 1 if J_I8 else N_DVE                 # k=0 via D0-pass in i8 mode
DCOLS = 2 * NCH * N_DVE_J                              # is_ge counts
ECOLS = 2 * NCH * 2                                    # E-pass + D0-pass (#x >= -1)
ACOLS = 2 * NHALF * N_ACT                              # sign sums
CPH = FH // FQ                                         # chunks per ACT half

_CACHE = {}


def _get_floorfix():
    """Register the custom fused DVE op once.

    Plain: j = Src1 - (Src0 < Src1)  (== floor(Src0) given Src1 = int-cast).
    i8 mode: j' = clamp(j - 128, -128, 127), safe for i8 output; saturation
    keeps out-of-range values on the correct side of all thresholds.
    """
    key = "ff_i8" if J_I8 else "ff"
    if key in _CACHE:
        return _CACHE[key]
    from concourse.dve_spec import Spec, Src0, Src1, lower, maxx, minn
    from concourse.dve_uop import DveOpSpec
    from concourse import dve_ops

    name = "ENT_FLOORFIX_I8" if J_I8 else "ENT_FLOORFIX"
    for existing in dve_ops.OPS:
        if existing.name == name:
            _CACHE[key] = existing
            return existing
    if J_I8:
        from concourse.dve_spec import C0, C1, C2
        spec = Spec(
            body=maxx(minn((Src1 - (Src0 < Src1)) - C0, C1), C2),
            reference=lambda in0, in1, s0, s1, imm2: np.maximum(
                np.minimum((in1 - (in0 < in1)) - s0, s1), imm2),
        )
    else:
        spec = Spec(
            body=Src1 - (Src0 < Src1),
            reference=lambda in0, in1, s0, s1, imm2: in1 - (in0 < in1),
        )
    row = dve_ops._CUSTOM_DVE_ROW_BASE + len(dve_ops.OPS)
    shas = {}
    for ver in ("v3", "v4"):
        tmp = DveOpSpec(name=name, opcode=row, uops=lower(spec, ver=ver),
                        rd1_en=True)
        shas[ver] = tmp.sha(ver)
    ff = dve_ops.DveOp(name, spec, subdim=False, uops_sha=shas)
    dve_ops.OPS.append(ff)
    dve_ops.CUSTOM_DVE_SPECS[name] = spec
    dve_ops._SUB_OPCODE_FOR_NAME[name] = row
    _CACHE[key] = ff
    return ff


def _build():
    import concourse.bacc as bacc
    import concourse.mybir as mybir
    import concourse.tile as tile

    f32 = mybir.dt.float32
    i16 = mybir.dt.int16
    i8 = mybir.dt.int8
    bf16 = mybir.dt.bfloat16
    op = mybir.AluOpType
    AF = mybir.ActivationFunctionType

    ff = _get_floorfix()

    nc = bacc.Bacc("TRN2", target_bir_lowering=False, debug=False,
                   num_devices=N_CORES)
    pred_d = nc.dram_tensor("pred", [P, FREE], f32, kind="ExternalInput")
    gt_d = nc.dram_tensor("gt", [P, FREE], f32, kind="ExternalInput")
    ktab_d = nc.dram_tensor("ktab", [P, N_ACT], f32, kind="ExternalInput")
    od_d = nc.dram_tensor("od", [P, DCOLS + ECOLS], f32, kind="ExternalOutput")
    oa_d = nc.dram_tensor("oa", [P, ACOLS], f32, kind="ExternalOutput")

    with tile.TileContext(nc) as tc:
        with (
            tc.tile_pool(name="xp", bufs=2) as xpool,
            tc.tile_pool(name="up", bufs=1) as upool,
            tc.tile_pool(name="rp", bufs=1) as rpool,
            tc.tile_pool(name="jp", bufs=4 if J_I8 else 2) as jpool,
            tc.tile_pool(name="tp", bufs=1) as tpool,
            tc.tile_pool(name="ap", bufs=1) as apool,
            tc.tile_pool(name="pp", bufs=1, space="PSUM") as ppool,
        ):
            ktab = apool.tile([P, N_ACT], f32)
            nc.sync.dma_start(ktab[:], ktab_d.ap())
            acc_d_sb = apool.tile([P, DCOLS + ECOLS], f32)
            acc_a_sb = apool.tile([P, ACOLS], f32)
            if D_PSUM:
                acc_d = ppool.tile([P, DCOLS + ECOLS], f32, tag="acc_d_ps")
            else:
                acc_d = acc_d_sb
            if A_PSUM:
                acc_a = ppool.tile([P, ACOLS], f32, tag="acc_a_ps")
            else:
                acc_a = acc_a_sb
            trash_d = tpool.tile([P, FQ], i8, tag="td")
            trash_a = tpool.tile([P, FH], i8, tag="ta")

            for t_i, src in ((0, pred_d), (1, gt_d)):
                # phase 1: prep all chunks of this tensor (keeps both engine
                # queues short so phase-2 counting fully overlaps; counting
                # reads the j halves, so u/r tiles die right after the
                # floor-fix and single buffers suffice)
                j_halves = []
                for c in range(NCH):
                    tc_i = t_i * NCH + c
                    lo = c * FQ
                    xs = xpool.tile([P, FQ], f32, tag="x")
                    nc.sync.dma_start(xs[:], src.ap()[:, lo:lo + FQ])
                    # E-pass: #{x >= 1+2^-23}  (= #{x > 1})
                    nc.vector.tensor_scalar(
                        trash_d[:], xs[:], X_ABOVE_ONE, None, op.is_ge, op.add,
                        accum_out=acc_d[:, DCOLS + tc_i:DCOLS + tc_i + 1])
                    if J_I8:
                        # D0-pass: D_0 = #{u >= 0} = #{x >= -1}
                        nc.vector.tensor_scalar(
                            trash_d[:], xs[:], -1.0, None, op.is_ge, op.add,
                            accum_out=acc_d[:, DCOLS + 2 * NCH + tc_i:
                                            DCOLS + 2 * NCH + tc_i + 1])
                    # u = (x + 1) * 128, fp32 (== fl(x+1)*128 exactly)
                    u = upool.tile([P, FQ], f32, tag="u")
                    nc.vector.tensor_scalar(
                        u[:], xs[:], 1.0, 128.0, op.add, op.mult)
                    # r = round-to-nearest-int16(u) on ScalarE
                    r = rpool.tile([P, FQ], i16, tag="r")
                    nc.scalar.activation(r[:], u[:], AF.Copy)
                    # j = r - (u < r) == floor(u); exact-integer halves.
                    # i8 mode stores clamp(j-128, -128, 127): high-saturation
                    # is on the correct side of every threshold; low clamp
                    # (-128) collides only with the k=0 boundary, which is
                    # counted by the D0-pass above instead.
                    if c % CPH == 0:
                        j_half = jpool.tile([P, FH], i8 if J_I8 else bf16,
                                            tag="j")
                        j_halves.append(j_half)
                    jslice = j_half[:, (c % CPH) * FQ:(c % CPH) * FQ + FQ]
                    if J_I8:
                        nc.vector._custom_dve(ff, out=jslice, in0=u[:],
                                              in1=r[:], s0=128.0, s1=127.0,
                                              imm2=-128.0)
                    else:
                        nc.vector._custom_dve(ff, out=jslice, in0=u[:], in1=r[:])
                # phase 2: counting; DVE on 8k j-slices, ACT on 16k halves.
                # Queues are independent here, so the engines free-run.
                for c in range(NCH):
                    tc_i = t_i * NCH + c
                    jsl = j_halves[c // CPH][:, (c % CPH) * FQ:
                                             (c % CPH) * FQ + FQ]
                    koff = (1, 128.0) if J_I8 else (0, 0.0)
                    for q in range(N_DVE_J):
                        k = q + koff[0]
                        col = tc_i * N_DVE_J + q
                        nc.vector.tensor_scalar(
                            trash_d[:], jsl, float(k) - koff[1] - 0.5, None,
                            op.is_ge, op.add,
                            accum_out=acc_d[:, col:col + 1])
                for h in range(NHALF):
                    h_i = t_i * NHALF + h
                    for i in range(N_ACT):
                        col = h_i * N_ACT + i
                        nc.scalar.activation(
                            trash_a[:], j_halves[h][:], AF.Sign,
                            bias=ktab[:, i:i + 1], scale=1.0,
                            accum_out=acc_a[:, col:col + 1])
            if D_PSUM:
                nc.vector.tensor_copy(acc_d_sb[:], acc_d[:])
            if A_PSUM:
                nc.scalar.copy(acc_a_sb[:], acc_a[:])
            nc.sync.dma_start(od_d.ap(), acc_d_sb[:])
            nc.sync.dma_start(oa_d.ap(), acc_a_sb[:])
    nc.compile()
    return nc


def _get_nc():
    if "nc" not in _CACHE:
        _CACHE["nc"] = _build()
    return _CACHE["nc"]


def _ktab():
    # ACT bias for boundary k = N_DVE + i:  -(k - 0.5), shifted by 128 in i8 mode
    ks = np.arange(N_DVE, NB, dtype=np.float64)
    off = 128.0 if J_I8 else 0.0
    return np.tile(-(ks - off - 0.5).astype(np.float32), (P, 1))


def _shard(arr):
    a = np.ascontiguousarray(np.asarray(arr, dtype=np.float32))
    per = B // N_CORES
    return [a[i * per:(i + 1) * per].reshape(P, FREE) for i in range(N_CORES)]


def _entropy_diff_from_hists(hp, hg):
    """Mirror reference._entropy in float32 on CPU via jax."""
    import jax
    import jax.numpy as jnp

    cpu = jax.devices("cpu")[0]
    with jax.default_device(cpu):
        def ent(h):
            h = jnp.asarray(np.asarray(h, dtype=np.float32))
            prob = h / jnp.sum(h) + np.float32(EPS)
            return -jnp.sum(prob * jnp.log(prob))
        out = jnp.abs(ent(hp) - ent(hg))
        return np.asarray(out).astype(np.float32).reshape(())


def kernel(predicted_ab, ground_truth_ab):
    from concourse import bass_utils

    nc = _get_nc()
    preds = _shard(predicted_ab)
    gts = _shard(ground_truth_ab)
    ktab = _ktab()
    in_maps = [{"pred": preds[i], "gt": gts[i], "ktab": ktab}
               for i in range(N_CORES)]
    res = bass_utils.run_bass_kernel_spmd(nc, in_maps,
                                          core_ids=list(range(N_CORES)))

    # D[t, k] cumulative counts, E[t] = #{x > 1}
    D = np.zeros((2, NB + 1), dtype=np.int64)
    E = np.zeros(2, dtype=np.int64)
    for cidx in range(N_CORES):
        od = np.asarray(res.results[cidx]["od"], dtype=np.float64)
        oa = np.asarray(res.results[cidx]["oa"], dtype=np.float64)
        for t in range(2):
            for c in range(NCH):
                tc_i = t * NCH + c
                blk = od[:, tc_i * N_DVE_J:(tc_i + 1) * N_DVE_J]
                k0 = 1 if J_I8 else 0
                D[t, k0:k0 + N_DVE_J] += blk.sum(axis=0).round().astype(np.int64)
                E[t] += int(od[:, DCOLS + tc_i].sum().round())
                if J_I8:
                    D[t, 0] += int(od[:, DCOLS + 2 * NCH + tc_i].sum().round())
            for h in range(NHALF):
                h_i = t * NHALF + h
                sgn = oa[:, h_i * N_ACT:(h_i + 1) * N_ACT]
                s = sgn.round().astype(np.int64)
                tot = (FH + s)
                assert np.all(tot % 2 == 0), "sign parity violated"
                D[t, N_DVE:NB] += (tot // 2).sum(axis=0)
    hist = np.zeros((2, NB), dtype=np.int64)
    for t in range(2):
        hist[t, :NB - 1] = D[t, :NB - 1] - D[t, 1:NB]
        hist[t, NB - 1] = D[t, NB - 1] - E[t]
    return _entropy_diff_from_hists(hist[0], hist[1])


if __name__ == "__main__":
    rng = np.random.default_rng(0)
    p = rng.standard_normal((B, C, H, W)).astype(np.float32)
    g = rng.standard_normal((B, C, H, W)).astype(np.float32)
    got = kernel(p, g)

    def host_hist(x):
        x = x.ravel()
        u = (x.astype(np.float32) + np.float32(1.0)) * np.float32(128.0)
        idx = np.clip(np.floor(u.astype(np.float64)).astype(np.int64), 0, 255)
        m = (x >= -1.0) & (x <= 1.0)
        return np.bincount(idx[m], minlength=256)

    hp, hg = host_hist(p), host_hist(g)
    exp = _entropy_diff_from_hists(hp, hg)
    print("kernel:", got, "host:", exp, "absdiff:", abs(float(got) - float(exp)))
